# revision 1
# baseline (speedup 1.0000x reference)
"""MinGRU Trainium2 kernel.

Full-input contract: kernel(x=[8,4096,1024] f32, W_hg=[2048,1024] f32)
-> [8,4096,1024] f32.

Sharding: data-parallel over batch. Core i computes example i entirely
(matmul + pointwise + sequential scan along seq); W_hg is replicated.

Math (equivalent to the log-space reference, but computed in linear space,
which is stable here because a_t = sigmoid(-gate) is in (0,1) and every
summand is positive):
    hg     = x @ W_hg.T ; hidden, gate = split(hg)
    a_t    = sigmoid(-gate_t)                        # 1 - z_t
    g~_t   = min(sigmoid(hidden_t), 0.5) + relu(hidden_t)
    b_t    = sigmoid(gate_t) * g~_t
    h_t    = a_t * h_{t-1} + b_t                     # tensor_tensor_scan

Device layout: channels on partitions, seq on the free dim. The host
pre-transposes x[i] -> xT [D, S] and W_hg -> wT [D, 2D] so no on-chip
transposes are needed and the fp32r matmul result lands scan-ready.

Measured on trn2 (marginal cost of extra For_i passes, min-of-12):
~210 us/pass per core -- at the PE fp32r streaming floor (~218 us for
16 e-blocks x 8 k-tiles x 4096 columns @ 2.4 GHz). A seq-chunk-outer
variant with W resident and x streamed modeled better (256 vs 286 us in
the cost model) but measured worse on hardware (293 us/pass), so this
d-block-outer, x-resident structure is kept.
"""

from contextlib import ExitStack

import numpy as np

B, S, D = 8, 4096, 1024
E = 2 * D
P = 128
KT = D // P  # contraction k-tiles
DB = D // P  # output-channel blocks
SC = 512  # seq chunk (PSUM bank = 512 f32)
NSC = S // SC

_NC_CACHE = {}


def _build_bass(repeat=1, loop_repeat=None, psum_bufs=2):
    import contextlib

    import concourse.tile as tile
    from concourse import bacc, mybir

    f32 = mybir.dt.float32
    f32r = mybir.dt.float32r
    AF = mybir.ActivationFunctionType
    OP = mybir.AluOpType

    nc = bacc.Bacc("TRN2", debug=False)
    xT = nc.dram_tensor("xT", [D, S], f32r, kind="ExternalInput").ap()
    wT = nc.dram_tensor("wT", [D, E], f32r, kind="ExternalInput").ap()
    out = nc.dram_tensor("out", [D, S], f32, kind="ExternalOutput").ap()

    # [k, p, e] view of wT for one-shot strided weight-slice loads
    wT_k = wT.rearrange("(k p) e -> p k e", p=P)

    with tile.TileContext(nc) as tc, ExitStack() as ctx:
        xpool = ctx.enter_context(tc.tile_pool(name="x", bufs=1))
        wpool = ctx.enter_context(tc.tile_pool(name="w", bufs=2))
        ppool = ctx.enter_context(
            tc.tile_pool(name="ps", bufs=psum_bufs, space="PSUM")
        )
        spool = ctx.enter_context(tc.tile_pool(name="s", bufs=2))
        opool = ctx.enter_context(tc.tile_pool(name="o", bufs=4))

        loop_cm = (
            tc.For_i(0, loop_repeat, 1)
            if loop_repeat is not None
            else contextlib.nullcontext()
        )
        with loop_cm:
            for _rep in range(repeat):
                # x fully resident: 64 tiles [128, 512], loaded seq-chunk-major
                # so the first d-block's first matmuls start after ~2MB of DMA.
                xt = [[None] * NSC for _ in range(KT)]
                for sc in range(NSC):
                    for k in range(KT):
                        t = xpool.tile([P, SC], f32r, tag=f"x{k}_{sc}")
                        nc.sync.dma_start(
                            t[:], xT[k * P : (k + 1) * P, sc * SC : (sc + 1) * SC]
                        )
                        xt[k][sc] = t

                for db in range(DB):
                    eh = db * P  # hidden channel block
                    eg = D + db * P  # gate channel block
                    wh = wpool.tile([P, KT, P], f32r, tag="wh")
                    nc.sync.dma_start(wh[:], wT_k[:, :, eh : eh + P])
                    wg = wpool.tile([P, KT, P], f32r, tag="wg")
                    nc.sync.dma_start(wg[:], wT_k[:, :, eg : eg + P])

                    prev_o = None
                    for sc in range(NSC):
                        ph = ppool.tile([P, SC], f32, tag="ph")
                        pg = ppool.tile([P, SC], f32, tag="pg")
                        for k in range(KT):
                            nc.tensor.matmul(
                                ph[:],
                                wh[:, k, :],
                                xt[k][sc][:],
                                start=(k == 0),
                                stop=(k == KT - 1),
                            )
                        for k in range(KT):
                            nc.tensor.matmul(
                                pg[:],
                                wg[:, k, :],
                                xt[k][sc][:],
                                start=(k == 0),
                                stop=(k == KT - 1),
                            )

                        # ScalarE straight out of PSUM
                        a = spool.tile([P, SC], f32, tag="a")
                        nc.scalar.activation(a[:], pg[:], AF.Sigmoid, scale=-1.0)
                        z = spool.tile([P, SC], f32, tag="z")
                        nc.scalar.activation(z[:], pg[:], AF.Sigmoid)
                        sh = spool.tile([P, SC], f32, tag="sh")
                        nc.scalar.activation(sh[:], ph[:], AF.Sigmoid)
                        r = spool.tile([P, SC], f32, tag="r")
                        nc.scalar.activation(r[:], ph[:], AF.Relu)

                        # g~ = min(sigmoid(h), 0.5) + relu(h);  b = z * g~
                        gt = spool.tile([P, SC], f32, tag="gt")
                        nc.vector.scalar_tensor_tensor(
                            gt[:], sh[:], 0.5, r[:], op0=OP.min, op1=OP.add
                        )
                        b = spool.tile([P, SC], f32, tag="b")
                        nc.vector.tensor_mul(b[:], z[:], gt[:])

                        o = opool.tile([P, SC], f32, tag="o")
                        init = 0.0 if sc == 0 else prev_o[:, SC - 1 : SC]
                        nc.vector.tensor_tensor_scan(
                            o[:], a[:], b[:], init, op0=OP.mult, op1=OP.add
                        )
                        prev_o = o
                        nc.sync.dma_start(
                            out[db * P : (db + 1) * P, sc * SC : (sc + 1) * SC],
                            o[:],
                        )
    nc.compile()
    return nc


def _get_nc():
    if "nc" not in _NC_CACHE:
        _NC_CACHE["nc"] = _build_bass()
    return _NC_CACHE["nc"]


def _run(in_maps, trace=False, **kw):
    from concourse import bass_utils

    nc = _get_nc()
    return bass_utils.run_bass_kernel_spmd(
        nc, in_maps, core_ids=list(range(B)), trace=trace, **kw
    )


def _make_in_maps(x, W_hg):
    x = np.ascontiguousarray(x, dtype=np.float32)
    wT = np.ascontiguousarray(W_hg.T, dtype=np.float32)
    return [
        {"xT": np.ascontiguousarray(x[i].T), "wT": wT} for i in range(B)
    ]


def kernel(x, W_hg):
    res = _run(_make_in_maps(x, W_hg))
    outs = [r["out"] for r in res.results]
    return np.stack([o.T for o in outs], axis=0).astype(np.float32)



# revision 4
# speedup vs baseline: 1.1911x; 1.1911x over previous
"""MinGRU Trainium2 kernel (v3: bf16 matmul, ACT/DVE-balanced pointwise,
split DMA rings).

Full-input contract: kernel(x=[8,4096,1024] f32, W_hg=[2048,1024] f32)
-> [8,4096,1024] f32.

Sharding: data-parallel over batch. Core i computes example i entirely;
W_hg is replicated.

Math (linear-space equivalent of the log-space reference):
    hg      = x @ W_hg.T ; hidden, gate = split(hg)
    a_t     = sigmoid(-gate_t)                       # = 1 - z_t
    g~_t    = max(sigmoid(hidden_t), hidden_t + 0.5) # exact identity
    h_t     = a_t h_{t-1} + (1 - a_t) g~_t
            = a_t h_{t-1} - bneg_t,  bneg_t = (a_t - 1) g~_t

Per (sc, db) tile ([128 channels, 512 seq]):
    PE   : 16 bf16 matmuls -> phg [128, 2, 512] f32 PSUM (2 banks:
           hidden | NEGATED gate; gate weight block negated on host so ONE
           combined sigmoid yields both s and a).
    ACT  : sa = sigmoid(phg)   [128,1024] -> bf16 (s | a)
    ACT  : hp = phg[:,0,:]+0.5 (Copy+bias) -> bf16      [gt_mode="act"]
    DVE  : gt = max(hp, s)     (tensor_tensor, bf16)
           -- or gt = (ph + 0.5) max s via one stt from PSUM [gt_mode="dve"]
    DVE  : bneg = (a - 1.0) * gt                     (stt)
    DVE  : o = scan(a, bneg, init, mult, subtract)   # o_t = a o_{t-1} - bneg
    DMA  : out[db, sc] <- o (bf16; host upcasts)

Loop order: sc OUTER, db inner -> scan carry (db, sc-1) is ready a full
sc block ahead; never on the critical path.

DMA: inputs (W, x) go on the ACT HWDGE ring, outputs on the SP ring, so
input prefetch is not FIFO-blocked behind output drains. x tiles are
reloaded right after their last use each pass (the data is loop-invariant
in the timing loop), so the reload for pass n+1 overlaps ~7/8 of pass n.

bf16 pipeline validated on CPU: Fro rel err ~2.9e-3 (gate: 2e-2).
"""

from contextlib import ExitStack

import numpy as np

B, S, D = 8, 4096, 1024
E = 2 * D
P = 128
KT = D // P  # contraction k-tiles
DB = D // P  # output-channel pair-blocks (hidden+gate pair per block)
SC = 512  # seq chunk (PSUM bank = 512 f32)
NSC = S // SC

_NC_CACHE = {}


def _build_bass(
    repeat=1,
    loop_repeat=None,
    psum_bufs=3,
    sa_bufs=4,
    work_bufs=4,
    gt_mode="act",  # "act": hp on ACT + max on DVE; "dve": stt on DVE
):
    import contextlib

    import concourse.tile as tile
    from concourse import bacc, mybir

    f32 = mybir.dt.float32
    bf16 = mybir.dt.bfloat16
    AF = mybir.ActivationFunctionType
    OP = mybir.AluOpType

    nc = bacc.Bacc("TRN2", debug=False)
    xT = nc.dram_tensor("xT", [D, S], bf16, kind="ExternalInput").ap()
    wT = nc.dram_tensor("wT", [D, E], bf16, kind="ExternalInput").ap()
    out = nc.dram_tensor("out", [D, S], bf16, kind="ExternalOutput").ap()

    xT_k = xT.rearrange("(k p) s -> p k s", p=P)
    wT_k = wT.rearrange("(k p) e -> p k e", p=P)

    with tile.TileContext(nc) as tc, ExitStack() as ctx:
        xpool = ctx.enter_context(tc.tile_pool(name="x", bufs=1))
        wpool = ctx.enter_context(tc.tile_pool(name="w", bufs=2))
        ppool = ctx.enter_context(
            tc.tile_pool(name="ps", bufs=psum_bufs, space="PSUM")
        )
        sapool = ctx.enter_context(tc.tile_pool(name="sa", bufs=sa_bufs))
        gpool = ctx.enter_context(tc.tile_pool(name="g", bufs=work_bufs))
        opool = ctx.enter_context(tc.tile_pool(name="o", bufs=2))

        # x tiles persist across For_i iterations; prologue-loaded once,
        # then re-loaded (same loop-invariant data) right after last use.
        xt = []
        for sc in range(NSC):
            t = xpool.tile([P, KT, SC], bf16, tag=f"x{sc}", name=f"xt{sc}")
            nc.scalar.dma_start(t[:], xT_k[:, :, sc * SC : (sc + 1) * SC])
            xt.append(t)

        loop_cm = (
            tc.For_i(0, loop_repeat, 1)
            if loop_repeat is not None
            else contextlib.nullcontext()
        )
        with loop_cm:
            for _rep in range(repeat):
                wt = wpool.tile([P, KT, E], bf16, tag="w")
                nc.scalar.dma_start(wt[:], wT_k)

                prev_o = [None] * DB
                for sc in range(NSC):
                    for db in range(DB):
                        eh = db * P
                        eg = D + db * P
                        phg = ppool.tile([P, 2, SC], f32, tag="phg")
                        for k in range(KT):
                            nc.tensor.matmul(
                                phg[:, 0, :],
                                wt[:, k, eh : eh + P],
                                xt[sc][:, k, :],
                                start=(k == 0),
                                stop=(k == KT - 1),
                            )
                        for k in range(KT):
                            nc.tensor.matmul(
                                phg[:, 1, :],
                                wt[:, k, eg : eg + P],
                                xt[sc][:, k, :],
                                start=(k == 0),
                                stop=(k == KT - 1),
                            )

                        # one sigmoid over both banks: s | a
                        sa = sapool.tile([P, 2, SC], bf16, tag="sa")
                        nc.scalar.activation(sa[:], phg[:], AF.Sigmoid)
                        s = sa[:, 0, :]
                        a = sa[:, 1, :]

                        gt = gpool.tile([P, SC], bf16, tag="gt")
                        if gt_mode == "act":
                            hp = gpool.tile([P, SC], bf16, tag="hp")
                            nc.scalar.activation(
                                hp[:], phg[:, 0, :], AF.Copy, bias=0.5
                            )
                            nc.vector.tensor_tensor(
                                gt[:], hp[:], s, op=OP.max
                            )
                        else:
                            nc.vector.scalar_tensor_tensor(
                                gt[:], phg[:, 0, :], 0.5, s,
                                op0=OP.add, op1=OP.max,
                            )

                        bneg = gpool.tile([P, SC], bf16, tag="bneg")
                        nc.vector.scalar_tensor_tensor(
                            bneg[:], a, 1.0, gt[:],
                            op0=OP.subtract, op1=OP.mult,
                        )

                        o = opool.tile([P, SC], bf16, tag=f"o{db}")
                        init = (
                            0.0 if sc == 0
                            else prev_o[db][:, SC - 1 : SC]
                        )
                        nc.vector.tensor_tensor_scan(
                            o[:], a, bneg[:], init,
                            op0=OP.mult, op1=OP.subtract,
                        )
                        prev_o[db] = o
                        nc.sync.dma_start(
                            out[eh : eh + P, sc * SC : (sc + 1) * SC],
                            o[:],
                        )
                    # prefetch this sc tile for the next pass (same data)
                    nc.scalar.dma_start(
                        xt[sc][:], xT_k[:, :, sc * SC : (sc + 1) * SC]
                    )
    nc.compile()
    return nc


def _get_nc():
    if "nc" not in _NC_CACHE:
        _NC_CACHE["nc"] = _build_bass()
    return _NC_CACHE["nc"]


def _run(in_maps, trace=False, **kw):
    from concourse import bass_utils

    nc = _get_nc()
    return bass_utils.run_bass_kernel_spmd(
        nc, in_maps, core_ids=list(range(B)), trace=trace, **kw
    )


def _make_in_maps(x, W_hg):
    import ml_dtypes

    bf = ml_dtypes.bfloat16
    x = np.asarray(x, dtype=np.float32)
    wT = np.ascontiguousarray(np.asarray(W_hg, dtype=np.float32).T)
    wT[:, D:] *= -1.0  # negated gate block: sigmoid gives a = sigmoid(-g)
    wTb = wT.astype(bf)
    return [
        {"xT": np.ascontiguousarray(x[i].T).astype(bf), "wT": wTb}
        for i in range(B)
    ]


def kernel(x, W_hg):
    res = _run(_make_in_maps(x, W_hg))
    outs = [r["out"] for r in res.results]
    return np.stack(
        [o.astype(np.float32).T for o in outs], axis=0
    )


# revision 17
# speedup vs baseline: 1.2380x; 1.0393x over previous
"""MinGRU Trainium2 kernel (v3: bf16 matmul, ACT/DVE-balanced pointwise,
split DMA rings).

Full-input contract: kernel(x=[8,4096,1024] f32, W_hg=[2048,1024] f32)
-> [8,4096,1024] f32.

Sharding: data-parallel over batch. Core i computes example i entirely;
W_hg is replicated.

Math (linear-space equivalent of the log-space reference):
    hg      = x @ W_hg.T ; hidden, gate = split(hg)
    a_t     = sigmoid(-gate_t)                       # = 1 - z_t
    g~_t    = max(sigmoid(hidden_t), hidden_t + 0.5) # exact identity
    h_t     = a_t h_{t-1} + (1 - a_t) g~_t
            = a_t h_{t-1} - bneg_t,  bneg_t = (a_t - 1) g~_t

Per (sc, db) tile ([128 channels, 512 seq]):
    PE   : 16 bf16 matmuls -> phg [128, 2, 512] f32 PSUM (2 banks:
           hidden | NEGATED gate; gate weight block negated on host so ONE
           combined sigmoid yields both s and a).
    ACT  : sa = sigmoid(phg)   [128,1024] -> bf16 (s | a)
    ACT  : hp = phg[:,0,:]+0.5 (Copy+bias) -> bf16      [gt_mode="act"]
    DVE  : gt = max(hp, s)     (tensor_tensor, bf16)
           -- or gt = (ph + 0.5) max s via one stt from PSUM [gt_mode="dve"]
    DVE  : bneg = (a - 1.0) * gt                     (stt)
    DVE  : o = scan(a, bneg, init, mult, subtract)   # o_t = a o_{t-1} - bneg
    DMA  : out[db, sc] <- o (bf16; host upcasts)

Loop order: sc OUTER, db inner -> scan carry (db, sc-1) is ready a full
sc block ahead; never on the critical path.

DMA: inputs (W, x) go on the ACT HWDGE ring, outputs on the SP ring, so
input prefetch is not FIFO-blocked behind output drains. x tiles are
reloaded right after their last use each pass (the data is loop-invariant
in the timing loop), so the reload for pass n+1 overlaps ~7/8 of pass n.

bf16 pipeline validated on CPU: Fro rel err ~2.9e-3 (gate: 2e-2).
"""

from contextlib import ExitStack

import numpy as np

B, S, D = 8, 4096, 1024
E = 2 * D
P = 128
KT = D // P  # contraction k-tiles
DB = D // P  # output-channel pair-blocks (hidden+gate pair per block)
SC = 512  # seq chunk (PSUM bank = 512 f32)
NSC = S // SC

_NC_CACHE = {}


def _build_bass(
    repeat=1,
    loop_repeat=None,
    psum_bufs=4,
    sa_bufs=4,
    work_bufs=4,
    gt_mode="act",  # "act": hp on ACT + max on DVE; "dve": stt on DVE
    stages=5,  # ablation: 0=PE only, 1=+sig, 2=+hp, 3=+gt, 4=+bneg, 5=full
    stream_inputs=1,  # 0: prologue-load only (ablation); 1: reload per pass
    mm_order="db",  # "db": per-pair k-inner; "k": k-outer, 2 pairs/group
):
    import contextlib

    import concourse.tile as tile
    from concourse import bacc, mybir

    f32 = mybir.dt.float32
    bf16 = mybir.dt.bfloat16
    AF = mybir.ActivationFunctionType
    OP = mybir.AluOpType

    nc = bacc.Bacc("TRN2", debug=False)
    xT = nc.dram_tensor("xT", [D, S], bf16, kind="ExternalInput").ap()
    wT = nc.dram_tensor("wT", [D, E], bf16, kind="ExternalInput").ap()
    out = nc.dram_tensor("out", [D, S], bf16, kind="ExternalOutput").ap()

    xT_k = xT.rearrange("(k p) s -> p k s", p=P)
    wT_k = wT.rearrange("(k p) e -> p k e", p=P)

    with tile.TileContext(nc) as tc, ExitStack() as ctx:
        xpool = ctx.enter_context(tc.tile_pool(name="x", bufs=1))
        wpool = ctx.enter_context(tc.tile_pool(name="w", bufs=2))
        ppool = ctx.enter_context(
            tc.tile_pool(name="ps", bufs=psum_bufs, space="PSUM")
        )
        sapool = ctx.enter_context(tc.tile_pool(name="sa", bufs=sa_bufs))
        gpool = ctx.enter_context(tc.tile_pool(name="g", bufs=work_bufs))
        opool = ctx.enter_context(tc.tile_pool(name="o", bufs=2))

        # x tiles persist across For_i iterations; prologue-loaded once,
        # then re-loaded (same loop-invariant data) right after last use.
        xt = []
        for sc in range(NSC):
            t = xpool.tile([P, KT, SC], bf16, tag=f"x{sc}", name=f"xt{sc}")
            nc.scalar.dma_start(t[:], xT_k[:, :, sc * SC : (sc + 1) * SC])
            xt.append(t)

        loop_cm = (
            tc.For_i(0, loop_repeat, 1)
            if loop_repeat is not None
            else contextlib.nullcontext()
        )
        # W is persistent (one buffer); prologue-loaded monolithically, then
        # per-pass re-streamed in per-block slices right after each block's
        # last use, so no pass-head 4MB serialization.
        wt = wpool.tile([P, KT, E], bf16, tag="w")
        nc.scalar.dma_start(wt[:], wT_k)

        with loop_cm:
            for _rep in range(repeat):

                def pointwise(sc, db, phg, prev_o):
                    if stages < 1:
                        return
                    # one sigmoid over both banks: s | a
                    sa = sapool.tile([P, 2, SC], bf16, tag="sa", name="sa")
                    nc.scalar.activation(sa[:], phg[:], AF.Sigmoid)
                    s = sa[:, 0, :]
                    a = sa[:, 1, :]

                    gt = gpool.tile([P, SC], bf16, tag="gt", name="gt")
                    if gt_mode == "act":
                        if stages < 2:
                            return
                        hp = gpool.tile([P, SC], bf16, tag="hp", name="hp")
                        nc.scalar.activation(
                            hp[:], phg[:, 0, :], AF.Copy, bias=0.5
                        )
                        if stages < 3:
                            return
                        nc.vector.tensor_tensor(
                            gt[:], hp[:], s, op=OP.max
                        )
                    else:
                        if stages < 3:
                            return
                        nc.vector.scalar_tensor_tensor(
                            gt[:], phg[:, 0, :], 0.5, s,
                            op0=OP.add, op1=OP.max,
                        )

                    if stages < 4:
                        return
                    bneg = gpool.tile([P, SC], bf16, tag="bneg", name="bneg")
                    nc.vector.scalar_tensor_tensor(
                        bneg[:], a, 1.0, gt[:],
                        op0=OP.subtract, op1=OP.mult,
                    )

                    if stages < 5:
                        return
                    o = opool.tile([P, SC], bf16, tag=f"o{db}", name="o")
                    init = (
                        0.0 if sc == 0
                        else prev_o[db][:, SC - 1 : SC]
                    )
                    nc.vector.tensor_tensor_scan(
                        o[:], a, bneg[:], init,
                        op0=OP.mult, op1=OP.subtract,
                    )
                    prev_o[db] = o
                    nc.sync.dma_start(
                        out[db * P : db * P + P, sc * SC : (sc + 1) * SC],
                        o[:],
                    )

                def reload_w(db):
                    # re-stream this block's weight slices (same data) for
                    # the next pass, right after their last use
                    if not stream_inputs:
                        return
                    for base in (db * P, D + db * P):
                        nc.scalar.dma_start(
                            wt[:, :, base : base + P],
                            wT_k[:, :, base : base + P],
                        )

                prev_o = [None] * DB
                for sc in range(NSC):
                    if mm_order == "db":
                        for db in range(DB):
                            eh = db * P
                            eg = D + db * P
                            phg = ppool.tile([P, 2, SC], f32, tag="phg")
                            for k in range(KT):
                                nc.tensor.matmul(
                                    phg[:, 0, :],
                                    wt[:, k, eh : eh + P],
                                    xt[sc][:, k, :],
                                    start=(k == 0),
                                    stop=(k == KT - 1),
                                )
                            for k in range(KT):
                                nc.tensor.matmul(
                                    phg[:, 1, :],
                                    wt[:, k, eg : eg + P],
                                    xt[sc][:, k, :],
                                    start=(k == 0),
                                    stop=(k == KT - 1),
                                )
                            pointwise(sc, db, phg, prev_o)
                            if sc == NSC - 1:
                                reload_w(db)
                    else:
                        # k-outer: each x k-slice streams 4 consecutive MMs
                        for g in range(DB // 2):
                            dbs = (2 * g, 2 * g + 1)
                            tiles = [
                                ppool.tile(
                                    [P, 2, SC], f32, tag="phg", name="phg"
                                )
                                for _ in dbs
                            ]
                            for k in range(KT):
                                xk = xt[sc][:, k, :]
                                for ti, db in enumerate(dbs):
                                    for half, base in (
                                        (0, db * P),
                                        (1, D + db * P),
                                    ):
                                        nc.tensor.matmul(
                                            tiles[ti][:, half, :],
                                            wt[:, k, base : base + P],
                                            xk,
                                            start=(k == 0),
                                            stop=(k == KT - 1),
                                        )
                            for ti, db in enumerate(dbs):
                                pointwise(sc, db, tiles[ti], prev_o)
                                if sc == NSC - 1:
                                    reload_w(db)
                    # prefetch this sc tile for the next pass (same data),
                    # one DMA per k-slice so no consumer waits on a 1MB
                    # transfer
                    if stream_inputs:
                        for k in range(KT):
                            nc.scalar.dma_start(
                                xt[sc][:, k, :],
                                xT_k[:, k, sc * SC : (sc + 1) * SC],
                            )
    nc.compile()
    return nc


def _get_nc():
    if "nc" not in _NC_CACHE:
        _NC_CACHE["nc"] = _build_bass()
    return _NC_CACHE["nc"]


def _run(in_maps, trace=False, **kw):
    from concourse import bass_utils

    nc = _get_nc()
    return bass_utils.run_bass_kernel_spmd(
        nc, in_maps, core_ids=list(range(B)), trace=trace, **kw
    )


def _make_in_maps(x, W_hg):
    import ml_dtypes

    bf = ml_dtypes.bfloat16
    x = np.asarray(x, dtype=np.float32)
    wT = np.ascontiguousarray(np.asarray(W_hg, dtype=np.float32).T)
    wT[:, D:] *= -1.0  # negated gate block: sigmoid gives a = sigmoid(-g)
    wTb = wT.astype(bf)
    return [
        {"xT": np.ascontiguousarray(x[i].T).astype(bf), "wT": wTb}
        for i in range(B)
    ]


def kernel(x, W_hg):
    res = _run(_make_in_maps(x, W_hg))
    outs = [r["out"] for r in res.results]
    return np.stack(
        [o.astype(np.float32).T for o in outs], axis=0
    )
